# revision 6
# baseline (speedup 1.0000x reference)
"""Trainium2 Bass kernel for nn_Attention_48661979463892.

Multi-head attention: B=2, H=8, dk=dv=64, T=S=2048, E=512.
  keys    = Wk @ x[b]          -> per head [64, T]
  values  = Wv @ x[b]          -> per head [64, T]
  queries = Wq @ y[b]          -> per head [64, S]
  scores  = keys^T @ queries + mask            [T, S]
  attn    = softmax(0.125 * scores, axis=T)    (normalize over keys axis)
  out     = values @ attn                      [64, S]
  res     = W @ concat_heads(out) + b          -> [B, S, O]

Sharding: 16 (batch, head) pairs over 8 cores -> core c handles batch c//4,
head-pair c%4 (global head rows 128*(c%4) .. +128).  Each core emits a
partial [S, O] contribution of the final linear (its 128 v-channels); the
host sums 4 partials per batch and adds the bias.

Per-core schedule (one s-chunk of 512 per outer step, ACT-bound steady
state at ~16.5us/chunk):
  - scores tile [t_tile=128, 2 heads x 512] in PSUM, one Exp per tile on
    the scalar engine (the pace-setting engine), AV accumulated over the
    16 t-tiles with a ones-column appended to values^T for the softmax
    denominator (M=65 AV matmul).
  - epilogue stage 1 (end of each chunk): drain the two [65,512] AV
    accumulators into a head-stacked [128,512] f32 tile + a [2,512]
    colsum tile; reciprocal; broadcast the reciprocals to a [128,512]
    PSUM tile with a single K=2 indicator matmul; one tensor_tensor mult
    produces the normalized bf16 attention output (lhsT of the final
    linear).
  - epilogue stage 2 (spread over tiles 2..5 of the NEXT chunk): four
    K=128 stacked-head final-linear matmuls + drains + stores on
    rotating DMA queues.  Only the last chunk's epilogue is exposed.
  - query projections for chunks 2/3 are emitted just-in-time inside
    chunks 1/2; values^T projections are interleaved into chunk 0.
Startup: inputs arrive j-slice-granular over 4 DMA queues so the keys
projection streams as x lands; a few dummy matmuls keep the PE p-state
ramped while the first slices arrive.
"""

import numpy as np

N_CORES = 8
B, I, T, S, O = 2, 512, 2048, 2048, 512
H_PER_CORE = 2
DK = 64
SCALING = DK ** -0.5  # 0.125

MM_DTYPE = "bf16"
N_WARMUP_MM = 6

_BUILD_CACHE = {}


def _split_multi_waits(nc):
    """walrus in this toolchain accepts only ONE sync wait per instruction.
    Hoist extra waits onto same-engine NoOps inserted just before."""
    import concourse.mybir as mybir

    ctr = 0
    for fn in nc.m.functions:
        for blk in fn.blocks:
            new_insts = []
            for inst in blk.instructions:
                si = inst.sync_info
                if si is not None and len(si.on_wait) > 1:
                    waits = list(si.on_wait)
                    for w in waits[:-1]:
                        ctr += 1
                        nop = mybir.InstNoOp(
                            name=f"waitsplit-{ctr}", ins=[], outs=[]
                        )
                        nop.engine = inst.engine
                        nop.sync_info = mybir.SyncInfo(on_wait=[w], on_update=[])
                        new_insts.append(nop)
                    del si.on_wait[:-1]
                new_insts.append(inst)
            blk.instructions[:] = new_insts


def _build(with_mask):
    import concourse.bass as bass
    import concourse.mybir as mybir
    import concourse.tile as tile
    from concourse.bass import ts, ds

    f32 = mybir.dt.float32
    mmdt = {
        "f32": f32,
        "f32r": mybir.dt.float32r,
        "bf16": mybir.dt.bfloat16,
    }[MM_DTYPE]
    nc = bass.Bass()
    x_p = nc.declare_dram_parameter("x4", [4, 128, T], mmdt, isOutput=False)
    y_p = nc.declare_dram_parameter("y4", [4, 128, 4, 512], mmdt, isOutput=False)
    wk_p = nc.declare_dram_parameter("wkT", [128, 4, 128], mmdt, isOutput=False)
    wv_p = nc.declare_dram_parameter("wvT", [128, 4, 128], mmdt, isOutput=False)
    wq_p = nc.declare_dram_parameter("wqT", [128, 4, 128], mmdt, isOutput=False)
    wc_p = nc.declare_dram_parameter("wcT", [128, O], mmdt, isOutput=False)
    if with_mask:
        mask_p = nc.declare_dram_parameter("maskT", [16, 128, S], f32, isOutput=False)
    res_p = nc.declare_dram_parameter("res", [S, O], f32, isOutput=True)

    N_SC = S // 512    # s chunks of 512
    N_TT = T // 128    # t tiles of 128

    with tile.TileContext(nc) as tc:
        with (
            nc.allow_low_precision(reason="bf16 matmul operands"),
            tc.tile_pool(name="consts", bufs=1) as consts,
            tc.tile_pool(name="exps", bufs=4) as exps_pool,
            tc.tile_pool(name="epi", bufs=2) as epi_pool,
            tc.tile_pool(name="osb", bufs=2) as osb_pool,
            tc.tile_pool(name="resout", bufs=4) as res_pool,
            tc.tile_pool(name="ps_scores", bufs=2, space="PSUM") as ps_scores_pool,
            tc.tile_pool(name="ps_acc", bufs=2, space="PSUM") as ps_acc_pool,
            tc.tile_pool(name="ps_misc", bufs=2, space="PSUM") as ps_misc_pool,
        ):
            # dummy matmuls on scratch data keep the PE p-state ramped while
            # the first input DMAs land
            scratch_sb = consts.tile([128, 512], mmdt)
            nc.vector.memset(scratch_sb, 0.0)
            for w in range(N_WARMUP_MM):
                ps_w = ps_scores_pool.tile([128, 1024], f32, tag="ps_s", name="ps_w")
                nc.tensor.matmul(
                    ps_w[:, 0:512], scratch_sb[:, 0:128], scratch_sb,
                    start=True, stop=True,
                )

            # ---------------- load inputs ----------------
            wk_sb = consts.tile([128, 4, 128], mmdt)
            wv_sb = consts.tile([128, 4, 128], mmdt)
            wq_sb = consts.tile([128, 4, 128], mmdt)
            wc_sb = consts.tile([128, O], mmdt)
            x_sb = consts.tile([128, 4, T], mmdt)
            y_sb = consts.tile([128, 4, S], mmdt)
            # weights on the gpsimd queue (small, needed first); x j-slices
            # split over sync/scalar/vector so the keys projection can
            # consume them in arrival order; y chunk 0 right behind.
            nc.gpsimd.dma_start(out=wk_sb, in_=wk_p[:, :, :])
            nc.sync.dma_start(out=x_sb[:, 0, :], in_=x_p[0])
            nc.gpsimd.dma_start(out=x_sb[:, 1, :], in_=x_p[1])
            nc.scalar.dma_start(out=x_sb[:, 2, :], in_=x_p[2])
            nc.sync.dma_start(out=x_sb[:, 3, :], in_=x_p[3])
            nc.gpsimd.dma_start(out=wq_sb, in_=wq_p[:, :, :])
            nc.scalar.dma_start(out=y_sb[:, :, 0:512], in_=y_p[0])
            nc.gpsimd.dma_start(out=wv_sb, in_=wv_p[:, :, :])
            nc.gpsimd.dma_start(out=wc_sb, in_=wc_p[:, :])
            nc.sync.dma_start(out=y_sb[:, :, ts(1, 512)], in_=y_p[1])
            nc.gpsimd.dma_start(out=y_sb[:, :, ts(2, 512)], in_=y_p[2])
            nc.sync.dma_start(out=y_sb[:, :, ts(3, 512)], in_=y_p[3])

            # ones row for the reciprocal broadcast matmuls (K=1, M=64)
            ones_sb = consts.tile([1, 64], mmdt)
            nc.vector.memset(ones_sb, 1.0)

            # ---------------- projections ----------------
            keys_sb = consts.tile([128, T], mmdt)
            qs_sb = consts.tile([128, S], mmdt)

            def project2(dst, w_sb, src, n0, fillers=0):
                """project n-slices n0 and n0+1 with the j loop outermost so
                each contraction chunk is consumed as its DMA lands."""
                ps0 = ps_misc_pool.tile([128, 512], f32, tag="misc", name="ps0")
                ps1 = ps_misc_pool.tile([128, 512], f32, tag="misc", name="ps1")
                for j in range(4):
                    for ps, n in ((ps0, n0), (ps1, n0 + 1)):
                        nc.tensor.matmul(
                            ps,
                            w_sb[:, j, :],
                            src[:, j, ts(n, 512)],
                            start=(j == 0),
                            stop=(j == 3),
                        )
                    if j < 3:
                        for w in range(fillers):
                            ps_w = ps_scores_pool.tile(
                                [128, 1024], f32, tag="ps_s", name="ps_w"
                            )
                            nc.tensor.matmul(
                                ps_w[:, 0:512], scratch_sb[:, 0:128], scratch_sb,
                                start=True, stop=True,
                            )
                nc.vector.tensor_copy(out=dst[:, ts(n0, 512)], in_=ps0)
                nc.vector.tensor_copy(out=dst[:, ts(n0 + 1, 512)], in_=ps1)

            def project1(dst, w_sb, src, n):
                ps = ps_misc_pool.tile([128, 512], f32, tag="misc", name="psj")
                for j in range(4):
                    nc.tensor.matmul(
                        ps,
                        w_sb[:, j, :],
                        src[:, j, ts(n, 512)],
                        start=(j == 0),
                        stop=(j == 3),
                    )
                nc.vector.tensor_copy(out=dst[:, ts(n, 512)], in_=ps)

            # values^T with ones columns: [t_part=128, tt, 130]
            # cols 0:64 head0, col 64 ones, cols 65:129 head1, col 129 ones.
            valT_sb = consts.tile([128, N_TT, 130], mmdt)
            nc.vector.memset(valT_sb[:, :, 64:65], 1.0)
            nc.vector.memset(valT_sb[:, :, 129:130], 1.0)

            def valT_proj(tt):
                ps = ps_misc_pool.tile([128, 128], f32, tag="misc", name="psv")
                for j in range(4):
                    nc.tensor.matmul(
                        ps,
                        x_sb[:, j, ts(tt, 128)],
                        wv_sb[:, j, :],
                        start=(j == 0),
                        stop=(j == 3),
                    )
                nc.vector.tensor_copy(out=valT_sb[:, tt, 0:64], in_=ps[:, 0:64])
                nc.vector.tensor_copy(out=valT_sb[:, tt, 65:129], in_=ps[:, 64:128])

            project2(keys_sb, wk_sb, x_sb, 0, fillers=2)
            project2(qs_sb, wq_sb, y_sb, 0)
            project2(keys_sb, wk_sb, x_sb, 2)
            # qs slices 2 and 3 are projected just-in-time in chunks 1 / 2

            def extra_work(tt):
                if tt == 0:
                    valT_proj(0); valT_proj(1)
                elif tt + 1 < N_TT:
                    valT_proj(tt + 1)

            # ---------------- epilogue ----------------
            def epi_stage1(ps_o, sc):
                """drain the AV accumulators head-stacked, build the
                normalized bf16 attention output for the final linear."""
                last = sc == N_SC - 1
                osb = osb_pool.tile([128, 512], f32, tag="osb", name="osb")
                # colsums side by side: cols 0:512 head0, 512:1024 head1
                cs_sb = epi_pool.tile([1, 1024], f32, tag="cs", name="cs_sb")
                if last:
                    # ACT is idle after the final exp: split the drains
                    nc.scalar.copy(osb[0:64, :], ps_o[0][0:64, :])
                    nc.scalar.copy(cs_sb[0:1, 0:512], ps_o[0][64:65, :])
                else:
                    nc.vector.tensor_copy(out=osb[0:64, :], in_=ps_o[0][0:64, :])
                    nc.vector.tensor_copy(out=cs_sb[0:1, 0:512], in_=ps_o[0][64:65, :])
                nc.vector.tensor_copy(out=osb[64:128, :], in_=ps_o[1][0:64, :])
                nc.vector.tensor_copy(out=cs_sb[0:1, 512:1024], in_=ps_o[1][64:65, :])
                rec_sb = epi_pool.tile([1, 1024], mmdt, tag="rec", name="rec_sb")
                nc.vector.reciprocal(out=rec_sb, in_=cs_sb)
                rec_ps = ps_misc_pool.tile([128, 512], f32, tag="misc", name="rec_ps")
                nc.tensor.matmul(
                    rec_ps[0:64, :], ones_sb, rec_sb[0:1, 0:512],
                    start=True, stop=True,
                )
                nc.tensor.matmul(
                    rec_ps[64:128, :], ones_sb, rec_sb[0:1, 512:1024],
                    start=True, stop=True,
                )
                osc = osb_pool.tile([128, 512], mmdt, tag="osc", name="osc")
                nc.vector.tensor_tensor(osc, osb, rec_ps, mybir.AluOpType.mult)
                return osc

            def epi_stage2(sc, osc, st, q_eng, drain_eng):
                ps_r = ps_misc_pool.tile([128, 512], f32, tag="misc", name="ps_r")
                nc.tensor.matmul(
                    ps_r, osc[:, ts(st, 128)], wc_sb, start=True, stop=True
                )
                r_sb = res_pool.tile([128, O], f32, tag="r_sb", name="r_sb")
                if drain_eng is nc.scalar:
                    nc.scalar.copy(r_sb, ps_r)
                else:
                    drain_eng.tensor_copy(out=r_sb, in_=ps_r)
                q_eng.dma_start(
                    out=res_p[ds(sc * 512 + st * 128, 128), :], in_=r_sb
                )

            # ---------------- attention main loop ----------------
            def t_loop(sc, prev_osc):
                ps_o = [
                    ps_acc_pool.tile([65, 512], f32, tag="av", name=f"ps_o{h}")
                    for h in range(2)
                ]
                for tt in range(N_TT):
                    if sc == 0:
                        extra_work(tt)
                    if prev_osc is not None and 2 <= tt <= 5:
                        st = tt - 2
                        q_eng = nc.sync if st % 2 == 0 else nc.gpsimd
                        epi_stage2(sc - 1, prev_osc, st, q_eng, nc.vector)
                    if sc in (1, 2) and tt == 10:
                        project1(qs_sb, wq_sb, y_sb, sc + 1)
                    ps_s = ps_scores_pool.tile([128, 1024], f32, tag="ps_s", name="ps_s")
                    if with_mask:
                        m_sb = exps_pool.tile([128, 512], f32, tag="mask", name="m_sb")
                        nc.sync.dma_start(out=m_sb, in_=mask_p[tt][:, ts(sc, 512)])
                    for h in range(2):
                        nc.tensor.matmul(
                            ps_s[:, ts(h, 512)],
                            keys_sb[64 * h : 64 * h + 64, ts(tt, 128)],
                            qs_sb[64 * h : 64 * h + 64, ts(sc, 512)],
                            start=True,
                            stop=True,
                        )
                        if with_mask:
                            nc.vector.tensor_tensor(
                                ps_s[:, ts(h, 512)],
                                ps_s[:, ts(h, 512)],
                                m_sb,
                                mybir.AluOpType.add,
                            )
                    ex = exps_pool.tile([128, 1024], mmdt)
                    nc.scalar.activation(
                        out=ex,
                        in_=ps_s,
                        func=mybir.ActivationFunctionType.Exp,
                        scale=float(SCALING),
                    )
                    for h in range(2):
                        nc.tensor.matmul(
                            ps_o[h],
                            valT_sb[:, tt, 65 * h : 65 * h + 65],
                            ex[:, ts(h, 512)],
                            start=(tt == 0),
                            stop=(tt == N_TT - 1),
                        )
                return epi_stage1(ps_o, sc)

            prev_osc = None
            for sc in range(N_SC):
                prev_osc = t_loop(sc, prev_osc)
            # tail: last chunk's final linear + stores on parallel queues
            tail_q = [nc.sync, nc.gpsimd, nc.scalar, nc.sync]
            tail_d = [nc.scalar, nc.vector, nc.scalar, nc.vector]
            for st in range(4):
                epi_stage2(N_SC - 1, prev_osc, st, tail_q[st], tail_d[st])

    _split_multi_waits(nc)
    return nc


def _get_nc(with_mask):
    key = (with_mask, MM_DTYPE)
    if key not in _BUILD_CACHE:
        _BUILD_CACHE[key] = _build(with_mask)
    return _BUILD_CACHE[key]


def _mm_np_dtype():
    if MM_DTYPE == "bf16":
        import ml_dtypes
        return np.dtype(ml_dtypes.bfloat16)
    return np.dtype(np.float32)


def _make_in_maps(x, y, mask, Wk, Wv, Wq, W, with_mask):
    mdt = _mm_np_dtype()
    in_maps = []
    for c in range(N_CORES):
        bb, hp = divmod(c, 4)
        e_sl = slice(128 * hp, 128 * hp + 128)
        im = {
            "x4": np.ascontiguousarray(
                x[bb].reshape(4, 128, T).astype(mdt)
            ),
            "y4": np.ascontiguousarray(
                y[bb].reshape(4, 128, 4, 512).transpose(2, 1, 0, 3).astype(mdt)
            ),
            "wkT": np.ascontiguousarray(
                Wk[e_sl].T.reshape(4, 128, 128).transpose(1, 0, 2).astype(mdt)
            ),
            "wvT": np.ascontiguousarray(
                Wv[e_sl].T.reshape(4, 128, 128).transpose(1, 0, 2).astype(mdt)
            ),
            "wqT": np.ascontiguousarray(
                Wq[e_sl].T.reshape(4, 128, 128).transpose(1, 0, 2).astype(mdt)
            ),
            "wcT": np.ascontiguousarray(
                W[:, 128 * hp : 128 * hp + 128].T.astype(mdt)
            ),
        }
        if with_mask:
            im["maskT"] = np.ascontiguousarray(mask.reshape(16, 128, S))
        in_maps.append(im)
    return in_maps


def kernel(x, y, mask, Wk, Wv, Wq, W, b):
    from concourse.bass_utils import run_bass_kernel_spmd

    x = np.asarray(x, dtype=np.float32)
    y = np.asarray(y, dtype=np.float32)
    mask = np.asarray(mask, dtype=np.float32)
    Wk = np.asarray(Wk, dtype=np.float32)
    Wv = np.asarray(Wv, dtype=np.float32)
    Wq = np.asarray(Wq, dtype=np.float32)
    W = np.asarray(W, dtype=np.float32)
    b = np.asarray(b, dtype=np.float32)

    with_mask = bool(np.any(mask))
    nc = _get_nc(with_mask)
    in_maps = _make_in_maps(x, y, mask, Wk, Wv, Wq, W, with_mask)

    r = run_bass_kernel_spmd(nc, in_maps, core_ids=list(range(N_CORES)))
    parts = [r.results[c]["res"] for c in range(N_CORES)]
    out = np.stack(
        [
            parts[0] + parts[1] + parts[2] + parts[3],
            parts[4] + parts[5] + parts[6] + parts[7],
        ],
        axis=0,
    )
    out += b[None, None, :]
    return out.astype(np.float32)


# revision 13
# speedup vs baseline: 1.1131x; 1.1131x over previous
"""Trainium2 Bass kernel for nn_Attention_48661979463892.

Multi-head attention: B=2, H=8, dk=dv=64, T=S=2048, E=512.
  keys    = Wk @ x[b]          -> per head [64, T]
  values  = Wv @ x[b]          -> per head [64, T]
  queries = Wq @ y[b]          -> per head [64, S]
  scores  = keys^T @ queries + mask            [T, S]
  attn    = softmax(0.125 * scores, axis=T)    (normalize over keys axis)
  out     = values @ attn                      [64, S]
  res     = W @ concat_heads(out) + b          -> [B, S, O]

Sharding: 16 (batch, head) pairs over 8 cores -> core c handles batch c//4,
head-pair c%4 (global head rows 128*(c%4) .. +128).  Each core emits a
partial [S, O] contribution of the final linear (its 128 v-channels); the
host sums 4 partials per batch and adds the bias.

Per-core schedule (one s-chunk of 512 per outer step, ACT-bound steady
state at ~16.5us/chunk):
  - scores tile [t_tile=128, 2 heads x 512] in PSUM, one Exp per tile on
    the scalar engine (the pace-setting engine), AV accumulated over the
    16 t-tiles with a ones-column appended to values^T for the softmax
    denominator (M=65 AV matmul).
  - epilogue stage 1 (end of each chunk): drain the two [65,512] AV
    accumulators into a head-stacked [128,512] f32 tile + a [2,512]
    colsum tile; reciprocal; broadcast the reciprocals to a [128,512]
    PSUM tile with a single K=2 indicator matmul; one tensor_tensor mult
    produces the normalized bf16 attention output (lhsT of the final
    linear).
  - epilogue stage 2 (spread over tiles 2..5 of the NEXT chunk): four
    K=128 stacked-head final-linear matmuls + drains + stores on
    rotating DMA queues.  Only the last chunk's epilogue is exposed.
  - query projections for chunks 2/3 are emitted just-in-time inside
    chunks 1/2; values^T projections are interleaved into chunk 0.
Startup: inputs arrive j-slice-granular over 4 DMA queues so the keys
projection streams as x lands; a few dummy matmuls keep the PE p-state
ramped while the first slices arrive.
"""

import numpy as np

N_CORES = 8
B, I, T, S, O = 2, 512, 2048, 2048, 512
H_PER_CORE = 2
DK = 64
SCALING = DK ** -0.5  # 0.125

MM_DTYPE = "bf16"
N_WARMUP_MM = 6

_BUILD_CACHE = {}


def _split_multi_waits(nc):
    """walrus in this toolchain accepts only ONE sync wait per instruction.
    Hoist extra waits onto same-engine NoOps inserted just before."""
    import concourse.mybir as mybir

    ctr = 0
    for fn in nc.m.functions:
        for blk in fn.blocks:
            new_insts = []
            for inst in blk.instructions:
                si = inst.sync_info
                if si is not None and len(si.on_wait) > 1:
                    waits = list(si.on_wait)
                    for w in waits[:-1]:
                        ctr += 1
                        nop = mybir.InstNoOp(
                            name=f"waitsplit-{ctr}", ins=[], outs=[]
                        )
                        nop.engine = inst.engine
                        nop.sync_info = mybir.SyncInfo(on_wait=[w], on_update=[])
                        new_insts.append(nop)
                    del si.on_wait[:-1]
                new_insts.append(inst)
            blk.instructions[:] = new_insts


def _build(with_mask):
    import concourse.bass as bass
    import concourse.mybir as mybir
    import concourse.tile as tile
    from concourse.bass import ts, ds

    f32 = mybir.dt.float32
    mmdt = {
        "f32": f32,
        "f32r": mybir.dt.float32r,
        "bf16": mybir.dt.bfloat16,
    }[MM_DTYPE]
    nc = bass.Bass()
    x_p = nc.declare_dram_parameter("x4", [4, 128, T], mmdt, isOutput=False)
    # y4[n] = [128, 4j*512] contiguous per partition (full-rate DMA)
    y_p = nc.declare_dram_parameter("y4", [4, 128, 4, 512], mmdt, isOutput=False)
    wk_p = nc.declare_dram_parameter("wkT", [128, 4, 128], mmdt, isOutput=False)
    wv_p = nc.declare_dram_parameter("wvT", [128, 4, 128], mmdt, isOutput=False)
    wq_p = nc.declare_dram_parameter("wqT", [128, 4, 128], mmdt, isOutput=False)
    wc_p = nc.declare_dram_parameter("wcT", [128, O], mmdt, isOutput=False)
    if with_mask:
        mask_p = nc.declare_dram_parameter("maskT", [16, 128, S], f32, isOutput=False)
    res_p = nc.declare_dram_parameter("res", [S, O], f32, isOutput=True)

    N_SC = S // 512    # s chunks of 512
    N_TT = T // 128    # t tiles of 128

    with tile.TileContext(nc) as tc:
        with (
            nc.allow_low_precision(reason="bf16 matmul operands"),
            tc.tile_pool(name="consts", bufs=1) as consts,
            tc.tile_pool(name="exps", bufs=4) as exps_pool,
            tc.tile_pool(name="epi", bufs=2) as epi_pool,
            tc.tile_pool(name="osb", bufs=2) as osb_pool,
            tc.tile_pool(name="resout", bufs=4) as res_pool,
            tc.tile_pool(name="ps_scores", bufs=2, space="PSUM") as ps_scores_pool,
            tc.tile_pool(name="ps_acc", bufs=2, space="PSUM") as ps_acc_pool,
            tc.tile_pool(name="ps_misc", bufs=2, space="PSUM") as ps_misc_pool,
        ):
            # dummy matmuls on scratch data keep the PE p-state ramped while
            # the first input DMAs land
            scratch_sb = consts.tile([128, 512], mmdt)
            nc.vector.memset(scratch_sb, 0.0)
            for w in range(N_WARMUP_MM):
                ps_w = ps_scores_pool.tile([128, 1024], f32, tag="ps_s", name="ps_w")
                nc.tensor.matmul(
                    ps_w[:, 0:512], scratch_sb[:, 0:128], scratch_sb,
                    start=True, stop=True,
                )

            # ---------------- load inputs ----------------
            wk_sb = consts.tile([128, 4, 128], mmdt)
            wv_sb = consts.tile([128, 4, 128], mmdt)
            wq_sb = consts.tile([128, 4, 128], mmdt)
            wc_sb = consts.tile([128, O], mmdt)
            x_sb = consts.tile([128, 4, T], mmdt)
            y_sb = consts.tile([128, 4, 4, 512], mmdt)  # [p, n, j, 512]
            # 3 DMA queues (sync/gpsimd/scalar), ~150GB/s each, FIFO per
            # queue.  x is split into (j, half-T) 256KB blocks so the keys
            # projection's j-loop streams in arrival order; y chunk n lands
            # contiguously per partition.  scalar's queue is kept short so
            # the ACT engine is free once exps start.
            nc.sync.dma_start(out=wk_sb, in_=wk_p[:, :, :])
            nc.gpsimd.dma_start(out=wq_sb, in_=wq_p[:, :, :])
            nc.scalar.dma_start(out=y_sb[:, 0], in_=y_p[0])
            nc.sync.dma_start(out=x_sb[:, 0, 0:1024], in_=x_p[0][:, 0:1024])
            nc.gpsimd.dma_start(out=x_sb[:, 1, 0:1024], in_=x_p[1][:, 0:1024])
            nc.sync.dma_start(out=x_sb[:, 2, 0:1024], in_=x_p[2][:, 0:1024])
            nc.gpsimd.dma_start(out=x_sb[:, 3, 0:1024], in_=x_p[3][:, 0:1024])
            nc.sync.dma_start(out=wv_sb, in_=wv_p[:, :, :])
            nc.sync.dma_start(out=x_sb[:, 0, 1024:2048], in_=x_p[0][:, 1024:2048])
            nc.gpsimd.dma_start(out=x_sb[:, 1, 1024:2048], in_=x_p[1][:, 1024:2048])
            nc.sync.dma_start(out=x_sb[:, 2, 1024:2048], in_=x_p[2][:, 1024:2048])
            nc.gpsimd.dma_start(out=x_sb[:, 3, 1024:2048], in_=x_p[3][:, 1024:2048])
            nc.sync.dma_start(out=y_sb[:, 2], in_=y_p[2])
            nc.gpsimd.dma_start(out=y_sb[:, 1], in_=y_p[1])
            nc.gpsimd.dma_start(out=y_sb[:, 3], in_=y_p[3])
            nc.gpsimd.dma_start(out=wc_sb, in_=wc_p[:, :])

            # ones row for the reciprocal broadcast matmuls (K=1, M=64)
            ones_sb = consts.tile([1, 64], mmdt)
            nc.vector.memset(ones_sb, 1.0)

            # ---------------- projections ----------------
            keys_sb = consts.tile([128, T], mmdt)
            qs_sb = consts.tile([128, S], mmdt)

            def x_src(j, n):
                return x_sb[:, j, ts(n, 512)]

            def y_src(j, n):
                return y_sb[:, n, j, :]

            def project2(dst, w_sb, src, n0, fillers=0):
                """project n-slices n0 and n0+1 with the j loop outermost so
                each contraction chunk is consumed as its DMA lands."""
                ps0 = ps_misc_pool.tile([128, 512], f32, tag="misc", name="ps0")
                ps1 = ps_misc_pool.tile([128, 512], f32, tag="misc", name="ps1")
                for j in range(4):
                    for ps, n in ((ps0, n0), (ps1, n0 + 1)):
                        nc.tensor.matmul(
                            ps,
                            w_sb[:, j, :],
                            src(j, n),
                            start=(j == 0),
                            stop=(j == 3),
                        )
                    if j < 3:
                        for w in range(fillers):
                            ps_w = ps_scores_pool.tile(
                                [128, 1024], f32, tag="ps_s", name="ps_w"
                            )
                            nc.tensor.matmul(
                                ps_w[:, 0:512], scratch_sb[:, 0:128], scratch_sb,
                                start=True, stop=True,
                            )
                nc.vector.tensor_copy(out=dst[:, ts(n0, 512)], in_=ps0)
                nc.vector.tensor_copy(out=dst[:, ts(n0 + 1, 512)], in_=ps1)

            def project1(dst, w_sb, src, n):
                ps = ps_misc_pool.tile([128, 512], f32, tag="misc", name="psj")
                for j in range(4):
                    nc.tensor.matmul(
                        ps,
                        w_sb[:, j, :],
                        src(j, n),
                        start=(j == 0),
                        stop=(j == 3),
                    )
                nc.vector.tensor_copy(out=dst[:, ts(n, 512)], in_=ps)

            # values^T with ones columns: [t_part=128, tt, 130]
            # cols 0:64 head0, col 64 ones, cols 65:129 head1, col 129 ones.
            valT_sb = consts.tile([128, N_TT, 130], mmdt)
            nc.vector.memset(valT_sb[:, :, 64:65], 1.0)
            nc.vector.memset(valT_sb[:, :, 129:130], 1.0)

            def valT_proj(tt):
                ps = ps_misc_pool.tile([128, 128], f32, tag="misc", name="psv")
                for j in range(4):
                    nc.tensor.matmul(
                        ps,
                        x_sb[:, j, ts(tt, 128)],
                        wv_sb[:, j, :],
                        start=(j == 0),
                        stop=(j == 3),
                    )
                nc.vector.tensor_copy(out=valT_sb[:, tt, 0:64], in_=ps[:, 0:64])
                nc.vector.tensor_copy(out=valT_sb[:, tt, 65:129], in_=ps[:, 64:128])

            project2(keys_sb, wk_sb, x_src, 0, fillers=2)
            project2(qs_sb, wq_sb, y_src, 0)
            project2(keys_sb, wk_sb, x_src, 2)
            # qs slices 2 and 3 are projected just-in-time in chunks 1 / 2

            def extra_work(tt):
                if tt == 0:
                    valT_proj(0); valT_proj(1)
                elif tt + 1 < N_TT:
                    valT_proj(tt + 1)

            # ---------------- epilogue ----------------
            def epi_stage1a(ps_o, sc):
                """drain the AV accumulators (head-stacked) and the colsum
                rows; emitted right after the chunk's last AV matmul."""
                last = sc == N_SC - 1
                osb = osb_pool.tile([128, 512], f32, tag="osb", name="osb")
                # colsums side by side: cols 0:512 head0, 512:1024 head1
                cs_sb = epi_pool.tile([1, 1024], mmdt, tag="cs", name="cs_sb")
                if last:
                    # ACT is idle after the final exp: split the drains
                    nc.scalar.copy(osb[0:64, :], ps_o[0][0:64, :])
                    nc.scalar.copy(cs_sb[0:1, 0:512], ps_o[0][64:65, :])
                else:
                    nc.vector.tensor_copy(out=osb[0:64, :], in_=ps_o[0][0:64, :])
                    nc.vector.tensor_copy(out=cs_sb[0:1, 0:512], in_=ps_o[0][64:65, :])
                nc.vector.tensor_copy(out=osb[64:128, :], in_=ps_o[1][0:64, :])
                nc.vector.tensor_copy(out=cs_sb[0:1, 512:1024], in_=ps_o[1][64:65, :])
                return osb, cs_sb

            def epi_stage1b(osb, cs_sb):
                """broadcast the colsums over the 2x64 head partitions with
                two K=1 matmuls, then one fast reciprocal + one multiply
                produce the normalized bf16 final-linear lhsT."""
                cs_ps = ps_misc_pool.tile([128, 512], f32, tag="misc", name="cs_ps")
                nc.tensor.matmul(
                    cs_ps[0:64, :], ones_sb, cs_sb[0:1, 0:512],
                    start=True, stop=True,
                )
                nc.tensor.matmul(
                    cs_ps[64:128, :], ones_sb, cs_sb[0:1, 512:1024],
                    start=True, stop=True,
                )
                rec_sb = epi_pool.tile([128, 512], f32, tag="rec", name="rec_sb")
                nc.vector.reciprocal(out=rec_sb, in_=cs_ps)
                osc = osb_pool.tile([128, 512], mmdt, tag="osc", name="osc")
                nc.vector.tensor_tensor(osc, osb, rec_sb, mybir.AluOpType.mult)
                return osc

            def epi_stage2(sc, osc, st, q_eng, drain_eng):
                ps_r = ps_misc_pool.tile([128, 512], f32, tag="misc", name="ps_r")
                nc.tensor.matmul(
                    ps_r, osc[:, ts(st, 128)], wc_sb, start=True, stop=True
                )
                r_sb = res_pool.tile([128, O], f32, tag="r_sb", name="r_sb")
                if drain_eng is nc.scalar:
                    nc.scalar.copy(r_sb, ps_r)
                else:
                    drain_eng.tensor_copy(out=r_sb, in_=ps_r)
                q_eng.dma_start(
                    out=res_p[ds(sc * 512 + st * 128, 128), :], in_=r_sb
                )

            # ---------------- attention main loop ----------------
            def t_loop(sc, prev):
                """scores/exp/AV for chunk sc; the previous chunk's epilogue
                is interleaved at tile boundaries so its PE/DVE work hides
                inside this chunk's ACT-bound stream."""
                ps_o = [
                    ps_acc_pool.tile([65, 512], f32, tag="av", name=f"ps_o{h}")
                    for h in range(2)
                ]
                osc_prev = None
                for tt in range(N_TT):
                    if sc == 0:
                        extra_work(tt)
                    if prev is not None:
                        if tt == 2:
                            osc_prev = epi_stage1b(*prev)
                        elif 4 <= tt <= 7:
                            st = tt - 4
                            q_eng = nc.sync if st % 2 == 0 else nc.gpsimd
                            epi_stage2(sc - 1, osc_prev, st, q_eng, nc.vector)
                    if sc in (1, 2) and tt == 10:
                        project1(qs_sb, wq_sb, y_src, sc + 1)
                    ps_s = ps_scores_pool.tile([128, 1024], f32, tag="ps_s", name="ps_s")
                    if with_mask:
                        m_sb = exps_pool.tile([128, 512], f32, tag="mask", name="m_sb")
                        nc.sync.dma_start(out=m_sb, in_=mask_p[tt][:, ts(sc, 512)])
                    for h in range(2):
                        nc.tensor.matmul(
                            ps_s[:, ts(h, 512)],
                            keys_sb[64 * h : 64 * h + 64, ts(tt, 128)],
                            qs_sb[64 * h : 64 * h + 64, ts(sc, 512)],
                            start=True,
                            stop=True,
                        )
                        if with_mask:
                            nc.vector.tensor_tensor(
                                ps_s[:, ts(h, 512)],
                                ps_s[:, ts(h, 512)],
                                m_sb,
                                mybir.AluOpType.add,
                            )
                    ex = exps_pool.tile([128, 1024], mmdt)
                    nc.scalar.activation(
                        out=ex,
                        in_=ps_s,
                        func=mybir.ActivationFunctionType.Exp,
                        scale=float(SCALING),
                    )
                    for h in range(2):
                        nc.tensor.matmul(
                            ps_o[h],
                            valT_sb[:, tt, 65 * h : 65 * h + 65],
                            ex[:, ts(h, 512)],
                            start=(tt == 0),
                            stop=(tt == N_TT - 1),
                        )
                return epi_stage1a(ps_o, sc)

            prev = None
            for sc in range(N_SC):
                prev = t_loop(sc, prev)
            # tail: last chunk's normalize + final linear, stores spread
            # over all three DMA queues
            osc = epi_stage1b(*prev)
            tail_q = [nc.sync, nc.gpsimd, nc.scalar, nc.sync]
            tail_d = [nc.scalar, nc.vector, nc.scalar, nc.vector]
            for st in range(4):
                epi_stage2(N_SC - 1, osc, st, tail_q[st], tail_d[st])

    _split_multi_waits(nc)
    return nc


def _get_nc(with_mask):
    key = (with_mask, MM_DTYPE)
    if key not in _BUILD_CACHE:
        _BUILD_CACHE[key] = _build(with_mask)
    return _BUILD_CACHE[key]


def _mm_np_dtype():
    if MM_DTYPE == "bf16":
        import ml_dtypes
        return np.dtype(ml_dtypes.bfloat16)
    return np.dtype(np.float32)


def _make_in_maps(x, y, mask, Wk, Wv, Wq, W, with_mask):
    mdt = _mm_np_dtype()
    in_maps = []
    for c in range(N_CORES):
        bb, hp = divmod(c, 4)
        e_sl = slice(128 * hp, 128 * hp + 128)
        im = {
            "x4": np.ascontiguousarray(
                x[bb].reshape(4, 128, T).astype(mdt)
            ),
            "y4": np.ascontiguousarray(
                y[bb].reshape(4, 128, 4, 512).transpose(2, 1, 0, 3).astype(mdt)
            ),
            "wkT": np.ascontiguousarray(
                Wk[e_sl].T.reshape(4, 128, 128).transpose(1, 0, 2).astype(mdt)
            ),
            "wvT": np.ascontiguousarray(
                Wv[e_sl].T.reshape(4, 128, 128).transpose(1, 0, 2).astype(mdt)
            ),
            "wqT": np.ascontiguousarray(
                Wq[e_sl].T.reshape(4, 128, 128).transpose(1, 0, 2).astype(mdt)
            ),
            "wcT": np.ascontiguousarray(
                W[:, 128 * hp : 128 * hp + 128].T.astype(mdt)
            ),
        }
        if with_mask:
            im["maskT"] = np.ascontiguousarray(mask.reshape(16, 128, S))
        in_maps.append(im)
    return in_maps


def kernel(x, y, mask, Wk, Wv, Wq, W, b):
    from concourse.bass_utils import run_bass_kernel_spmd

    x = np.asarray(x, dtype=np.float32)
    y = np.asarray(y, dtype=np.float32)
    mask = np.asarray(mask, dtype=np.float32)
    Wk = np.asarray(Wk, dtype=np.float32)
    Wv = np.asarray(Wv, dtype=np.float32)
    Wq = np.asarray(Wq, dtype=np.float32)
    W = np.asarray(W, dtype=np.float32)
    b = np.asarray(b, dtype=np.float32)

    with_mask = bool(np.any(mask))
    nc = _get_nc(with_mask)
    in_maps = _make_in_maps(x, y, mask, Wk, Wv, Wq, W, with_mask)

    r = run_bass_kernel_spmd(nc, in_maps, core_ids=list(range(N_CORES)))
    parts = [r.results[c]["res"] for c in range(N_CORES)]
    out = np.stack(
        [
            parts[0] + parts[1] + parts[2] + parts[3],
            parts[4] + parts[5] + parts[6] + parts[7],
        ],
        axis=0,
    )
    out += b[None, None, :]
    return out.astype(np.float32)
